# revision 41
# baseline (speedup 1.0000x reference)
"""BD3LM block-diffusion decoder layer on 8 trn2 NeuronCores.

Sharding: core = 2*b + g  (b = batch 0..3, g = head-group 0..1, 8 heads each).
Each core: QKV projections for its batch/head-group, sparse BD3LM attention
(only ~80 of 256 score tiles per head), O-projection against its Wo row-slice.
Host: sums the two group partials per batch and adds the (bv @ Wo + bo)
correction (softmax rows sum to 1, so the v-bias contributes exactly bv @ Wo).

v3 structure:
  - everything bf16 on the PE (1 cyc/row at any free size; f32r is 4 cyc/row
    below N=256). PSUM accumulation stays f32. End-to-end error ~5e-3.
  - x resident in SBUF; phase A emitted PAIR-major (projections for head
    pair p over the full sequence), immediately followed by attention for
    those two heads. Attention is ACT(exp)-bound, projections are PE-bound:
    the Tile scheduler overlaps pair p's exps with pair p+1's projections.
  - per head a [*,128] stationary v-block: col 0 = ones (softmax denominator
    -> ctx row 0, where reciprocal_approx_fast + gpsimd partition_broadcast
    work - both require physical partition 0), cols 64..127 = v channels
    (ctx rows 64..127, base-64-aligned for the normalize multiply).
  - scores [k_tile, q_span] into [128,1024] PSUM; ONE exp per (head, half,
    j) span and one batched exp for the 8 block-diagonal tiles (ACT per-op
    overhead ~293ns).
  - PSUM budget: proj pool 2 banks + shared score/ctx pool 6 banks.
"""

import numpy as np

import concourse.bass as bass
import concourse.mybir as mybir
import concourse.tile as tile
from concourse import bacc
from concourse.bass_utils import run_bass_kernel_spmd

F32 = mybir.dt.float32
BF16 = mybir.dt.bfloat16
Act = mybir.ActivationFunctionType

B, T, D = 4, 2048, 1024
H, HD = 16, 64
L = T // 2           # 1024, length of each of [xt | x0]
BS = 4               # block size
G = 2                # head groups (cores per batch)
DG = D // G          # 512 channels per group
HG = H // G          # 8 heads per core
P = 128
NT = L // P          # 8 key/query tiles per half
KC = D // P          # 8 contraction chunks
DT4 = DG // P        # 4 output-partition tiles for qT/kT

REPEAT = 1  # loop whole computation inside the NEFF (timing experiments only)
DBG = False

_CACHE = {}


def _chunks512(a0, a1):
    """Split [a0, a1) at multiples of 512 (PSUM bank boundaries)."""
    out = []
    while a0 < a1:
        b1 = min(a1, (a0 // 512 + 1) * 512)
        out.append((a0, b1))
        a0 = b1
    return out


def _build():
    import concourse.tile_utils as tile_utils

    tile_utils.max_sbuf_usage = 204 * 1024

    nc = bacc.Bacc("TRN2", target_bir_lowering=False, debug=False, num_devices=8)

    xT = nc.dram_tensor("xT", [D, T], BF16, kind="ExternalInput").ap()
    wq = nc.dram_tensor("wq", [D, DG], BF16, kind="ExternalInput").ap()
    wk = nc.dram_tensor("wk", [D, DG], BF16, kind="ExternalInput").ap()
    wv = nc.dram_tensor("wv", [D, DG], BF16, kind="ExternalInput").ap()
    wo = nc.dram_tensor("wo", [DG, D], BF16, kind="ExternalInput").ap()
    bqs = nc.dram_tensor("bqs", [DG], F32, kind="ExternalInput").ap()
    bks = nc.dram_tensor("bks", [DG], F32, kind="ExternalInput").ap()
    msk = nc.dram_tensor("msk", [3, P, P], BF16, kind="ExternalInput").ap()
    out = nc.dram_tensor("out", [T, D], F32, kind="ExternalOutput").ap()

    dbg = {}
    if DBG:
        for nm, shp, dt in (
            ("dbg_qT", [P, DT4, T], BF16),
            ("dbg_kT", [P, DT4, T], BF16),
            ("dbg_v", [P, T // P, HG * 2 * HD], BF16),
            ("dbg_ctxT", [P, DT4, T], BF16),
        ):
            dbg[nm] = nc.dram_tensor(nm, shp, dt, kind="ExternalOutput").ap()

    views = dict(
        dbg=dbg,
        xT_v=xT.rearrange("(kc p) t -> p kc t", p=P),    # [128, 8, 2048]
        wq_v=wq.rearrange("(kc p) m -> p kc m", p=P),    # [128, 8, 512]
        wk_v=wk.rearrange("(kc p) m -> p kc m", p=P),
        wv_v=wv.rearrange("(kc p) m -> p kc m", p=P),
        wo_v=wo.rearrange("(cc p) n -> p cc n", p=P),    # [128, 4, 1024]
        msk=msk,
        out=out,
    )

    with tile.TileContext(nc) as tc:
        with tc.tile_pool(name="persist", bufs=1) as pers:
            st = dict(
                x_sb=pers.tile([P, KC, T], BF16, name="x_sb"),
                wq_sb=pers.tile([P, KC, DG], BF16, name="wq_sb"),
                wk_sb=pers.tile([P, KC, DG], BF16, name="wk_sb"),
                wv_sb=pers.tile([P, KC, DG], BF16, name="wv_sb"),
                wo_sb=pers.tile([P, DT4, D], BF16, name="wo_sb"),
                qT_sb=pers.tile([P, DT4, T], BF16, name="qT_sb"),
                kT_sb=pers.tile([P, DT4, T], BF16, name="kT_sb"),
                v_sb=pers.tile([P, T // P, HG * 2 * HD], BF16, name="v_sb"),
                ctxT_sb=pers.tile([P, DT4, T], BF16, name="ctxT_sb"),
                bq_sb=pers.tile([P, DT4], F32, name="bq_sb"),
                bk_sb=pers.tile([P, DT4], F32, name="bk_sb"),
                m_strict=pers.tile([P, P], BF16, name="m_strict"),
                m_incl=pers.tile([P, P], BF16, name="m_incl"),
                m_diag=pers.tile([P, P], BF16, name="m_diag"),
            )
            nc.sync.dma_start(st["bq_sb"], bqs.rearrange("(c p) -> p c", p=P))
            nc.sync.dma_start(st["bk_sb"], bks.rearrange("(c p) -> p c", p=P))
            nc.sync.dma_start(st["m_strict"], msk[0])
            nc.sync.dma_start(st["m_incl"], msk[1])
            nc.sync.dma_start(st["m_diag"], msk[2])
            nc.gpsimd.memset(st["v_sb"], 0.0)
            ones_v = st["v_sb"].rearrange("p t (h c) -> p (t h) c", c=2 * HD)[
                :, :, 0:1
            ]
            nc.vector.memset(ones_v, 1.0)

            for _rep in range(REPEAT):
                _phases(nc, tc, st, views)

    nc.compile()
    return nc


def _load(nc, st, views):
    """Stream weights + x into SBUF across both HWDGE engines (SP + ACT),
    (kc, slab)-split so the first matmuls unblock after ~1MB of traffic."""
    # critical pieces first (DMA queues are FIFO): x slab 0 + wq/wk pair 0,
    # kc-split 32KB chunks alternating across the two HWDGE engines
    for kc in range(KC):
        eng, eng2 = (nc.sync, nc.scalar) if kc % 2 == 0 else (nc.scalar, nc.sync)
        eng.dma_start(
            st["x_sb"][:, kc, 0:512], views["xT_v"][:, kc, 0:512]
        )
        eng2.dma_start(
            st["wq_sb"][:, kc, 0:P], views["wq_v"][:, kc, 0:P]
        )
    nc.scalar.dma_start(st["wk_sb"][:, :, 0:P], views["wk_v"][:, :, 0:P])
    nc.sync.dma_start(st["wv_sb"][:, :, 0:P], views["wv_v"][:, :, 0:P])
    for s in range(1, T // 512):
        for kc in range(KC):
            eng = nc.sync if (kc % 2 == 0) else nc.scalar
            eng.dma_start(
                st["x_sb"][:, kc, 512 * s : 512 * (s + 1)],
                views["xT_v"][:, kc, 512 * s : 512 * (s + 1)],
            )
    for d4 in range(1, DT4):
        nc.sync.dma_start(
            st["wq_sb"][:, :, P * d4 : P * (d4 + 1)],
            views["wq_v"][:, :, P * d4 : P * (d4 + 1)],
        )
        nc.scalar.dma_start(
            st["wk_sb"][:, :, P * d4 : P * (d4 + 1)],
            views["wk_v"][:, :, P * d4 : P * (d4 + 1)],
        )
        nc.sync.dma_start(
            st["wv_sb"][:, :, P * d4 : P * (d4 + 1)],
            views["wv_v"][:, :, P * d4 : P * (d4 + 1)],
        )
    nc.scalar.dma_start(st["wo_sb"], views["wo_v"])


def _proj_pair(nc, st, pp, p):
    """QK projections for head pair p (qT/kT column-tile p); all of v is
    emitted with pair 0 (x-stationary N=512 matmuls cover every head)."""
    x_sb, v_sb = st["x_sb"], st["v_sb"]
    for w_sb, b_key, dst, scale in (
        (st["wq_sb"], "bq_sb", st["qT_sb"], HD ** -0.5),
        (st["wk_sb"], "bk_sb", st["kT_sb"], 1.0),
    ):
        for s in range(T // 512):
            ps = pp.tile([P, 512], F32, tag="pp", name=f"pp{p}_{s}")
            for kc in range(KC):
                nc.tensor.matmul(
                    ps,
                    w_sb[:, kc, P * p : P * (p + 1)],
                    x_sb[:, kc, 512 * s : 512 * (s + 1)],
                    start=(kc == 0),
                    stop=(kc == KC - 1),
                )
            nc.scalar.activation(
                dst[:, p, 512 * s : 512 * (s + 1)],
                ps,
                Act.Identity,
                bias=st[b_key][:, p : p + 1],
                scale=scale,
            )
    for tt in range(T // P):
        ps = pp.tile([P, 512], F32, tag="pp", name=f"ppv{p}_{tt}")
        for kc in range(KC):
            nc.tensor.matmul(
                ps[:, :P],
                x_sb[:, kc, P * tt : P * (tt + 1)],
                st["wv_sb"][:, kc, P * p : P * (p + 1)],
                start=(kc == 0),
                stop=(kc == KC - 1),
            )
        nc.vector.tensor_copy(
            v_sb[:, tt].rearrange("p (h c) -> p h c", c=2 * HD)[
                :, 2 * p : 2 * p + 2, HD : 2 * HD
            ],
            ps[:, :P].rearrange("p (h c) -> p h c", c=HD),
        )


def _attn_head(nc, st, bpool, atpool, tmppool, h):
    """Sparse BD3LM attention + normalize for one head."""
    qT_sb, kT_sb, v_sb = st["qT_sb"], st["kT_sb"], st["v_sb"]
    c, p0 = h // 2, HD * (h % 2)
    qh = qT_sb[p0 : p0 + HD, c, :]
    kh = kT_sb[p0 : p0 + HD, c, :]
    for half in range(2):
        mask = st["m_strict"] if half == 0 else st["m_incl"]
        ctx = bpool.tile([P, L], F32, tag="ps2", name=f"ctx{h}_{half}")
        for j in range(NT):
            span0 = P * j
            n = L - span0
            sc = bpool.tile([P, 1024], F32, tag="ps2", name=f"sc{h}_{half}_{j}")
            kv = kh[:, L + span0 : L + span0 + P]
            for r0, r1 in _chunks512(0, n):
                nc.tensor.matmul(
                    sc[:, r0:r1],
                    kv,
                    qh[:, L * half + span0 + r0 : L * half + span0 + r1],
                    start=True,
                    stop=True,
                    tile_position=(p0, 0),
                )
            at = atpool.tile([P, 1024], BF16, tag="at", name=f"at{h}_{half}_{j}")
            nc.scalar.activation(at[:, :n], sc[:, :n], Act.Exp)
            nc.vector.tensor_mul(at[:, :P], at[:, :P], mask)
            vj = v_sb[:, NT + j, 2 * HD * h : 2 * HD * (h + 1)]
            for a0, a1 in _chunks512(span0, L):
                last = half == 1 and (
                    (a1 <= 512 and j == 3) or (a0 >= 512 and j == NT - 1)
                )
                nc.tensor.matmul(
                    ctx[:, a0:a1],
                    vj,
                    at[:, a0 - span0 : a1 - span0],
                    start=(j == 0),
                    stop=last,
                )
        if half == 0:
            # xt-xt block-diagonal tiles, batched exp + mask
            scd = bpool.tile([P, 1024], F32, tag="ps2", name=f"scd{h}")
            for i in range(NT):
                nc.tensor.matmul(
                    scd[:, P * i : P * (i + 1)],
                    kh[:, P * i : P * (i + 1)],
                    qh[:, P * i : P * (i + 1)],
                    start=True,
                    stop=True,
                    tile_position=(p0, 0),
                )
            atd = atpool.tile([P, 1024], BF16, tag="at", name=f"atd{h}")
            nc.scalar.activation(atd, scd, Act.Exp)
            nc.vector.tensor_mul(
                atd.rearrange("p (i q) -> p i q", q=P),
                atd.rearrange("p (i q) -> p i q", q=P),
                st["m_diag"][:, None, :].to_broadcast((P, NT, P)),
            )
            for i in range(NT):
                nc.tensor.matmul(
                    ctx[:, P * i : P * (i + 1)],
                    v_sb[:, i, 2 * HD * h : 2 * HD * (h + 1)],
                    atd[:, P * i : P * (i + 1)],
                    start=False,
                    stop=(i == 3 or i == NT - 1),
                )
        # normalize per 512-bank: bank0's denominator row is final while
        # j=4..7 (resp. diag 4..7) still accumulate bank1, so its
        # recip/broadcast/multiply chain overlaps the remaining matmuls.
        cs = tmppool.tile([P, L], BF16, tag="cs", bufs=3, name=f"cs{h}_{half}")
        for c0 in range(0, L, 512):
            recip = tmppool.tile(
                [1, 512], F32, tag="recip", bufs=4, name=f"rc{h}_{half}_{c0}"
            )
            nc.vector.reciprocal_approx_fast(recip, ctx[0:1, c0 : c0 + 512])
            rb = tmppool.tile(
                [P, 512], F32, tag="rb", bufs=4, name=f"rb{h}_{half}_{c0}"
            )
            nc.gpsimd.partition_broadcast(rb, recip, channels=P)
            nc.vector.tensor_mul(
                cs[HD:P, c0 : c0 + 512],
                ctx[HD:P, c0 : c0 + 512],
                rb[HD:P, :],
            )
            nc.sync.dma_start(
                st["ctxT_sb"][
                    p0 : p0 + HD, c, L * half + c0 : L * half + c0 + 512
                ],
                cs[HD:P, c0 : c0 + 512],
            )


def _phases(nc, tc, st, views):
    from contextlib import ExitStack as _ES

    _load(nc, st, views)

    with tc.tile_pool(name="tmppool", bufs=2) as tmppool:
        _es = _ES()
        atpool = _es.enter_context(tc.tile_pool(name="atpool", bufs=8))
        pp = _es.enter_context(tc.tile_pool(name="pp", bufs=2, space="PSUM"))
        bpool = _es.enter_context(tc.tile_pool(name="bpool", bufs=3, space="PSUM"))

        for p in range(DT4):
            _proj_pair(nc, st, pp, p)
            _attn_head(nc, st, bpool, atpool, tmppool, 2 * p)
            _attn_head(nc, st, bpool, atpool, tmppool, 2 * p + 1)

        if DBG:
            nc.sync.dma_start(views["dbg"]["dbg_qT"], st["qT_sb"])
            nc.sync.dma_start(views["dbg"]["dbg_kT"], st["kT_sb"])
            nc.sync.dma_start(views["dbg"]["dbg_v"], st["v_sb"])
            nc.sync.dma_start(views["dbg"]["dbg_ctxT"], st["ctxT_sb"])

        _es.close()

        # ---------------- Phase C: O-projection ----------------
        with tc.tile_pool(name="opsum", bufs=6, space="PSUM") as opsum:
            for tt in range(T // P):
                for nk in range(2):
                    ops = opsum.tile([P, 512], F32, tag="op", name=f"op{tt}_{nk}")
                    for cc in range(DT4):
                        nc.tensor.matmul(
                            ops,
                            st["ctxT_sb"][:, cc, P * tt : P * (tt + 1)],
                            st["wo_sb"][:, cc, 512 * nk : 512 * (nk + 1)],
                            start=(cc == 0),
                            stop=(cc == DT4 - 1),
                        )
                    osb = tmppool.tile(
                        [P, 512], F32, tag="osb", bufs=6, name=f"osb{tt}_{nk}"
                    )
                    nc.vector.tensor_copy(osb, ops)
                    nc.sync.dma_start(
                        views["out"][
                            P * tt : P * (tt + 1), 512 * nk : 512 * (nk + 1)
                        ],
                        osb,
                    )


def _masks():
    import ml_dtypes

    q = np.arange(P)[None, :] // BS
    k = np.arange(P)[:, None] // BS
    m = np.zeros((3, P, P), np.float32)
    m[0] = (q > k).astype(np.float32)    # strict (xt q vs x0 k, same tile)
    m[1] = (q >= k).astype(np.float32)   # incl (x0 q vs x0 k, same tile)
    m[2] = (q == k).astype(np.float32)   # diag (xt q vs xt k, same tile)
    return m.astype(ml_dtypes.bfloat16)


def kernel(x, Wq, bq, Wk, bk, Wv, bv, Wo, bo, block_size=4, **_):
    import ml_dtypes

    BF = ml_dtypes.bfloat16
    x = np.asarray(x, np.float32)
    Wq, bq = np.asarray(Wq, np.float32), np.asarray(bq, np.float32)
    Wk, bk = np.asarray(Wk, np.float32), np.asarray(bk, np.float32)
    Wv, bv = np.asarray(Wv, np.float32), np.asarray(bv, np.float32)
    Wo, bo = np.asarray(Wo, np.float32), np.asarray(bo, np.float32)

    if "nc" not in _CACHE:
        _CACHE["nc"] = _build()
    nc = _CACHE["nc"]

    masks = _masks()
    scale = HD ** -0.5
    in_maps = []
    for core in range(8):
        b, g = core // 2, core % 2
        cols = slice(DG * g, DG * (g + 1))
        in_maps.append(
            {
                "xT": np.ascontiguousarray(x[b].T).astype(BF),
                "wq": np.ascontiguousarray(Wq[:, cols]).astype(BF),
                "wk": np.ascontiguousarray(Wk[:, cols]).astype(BF),
                "wv": np.ascontiguousarray(Wv[:, cols]).astype(BF),
                "wo": np.ascontiguousarray(Wo[cols, :]).astype(BF),
                "bqs": np.ascontiguousarray(bq[cols]) * np.float32(scale),
                "bks": np.ascontiguousarray(bk[cols]),
                "msk": masks,
            }
        )

    _CACHE["last_in_maps"] = in_maps
    last_err = None
    for _attempt in range(6):
        try:
            res = run_bass_kernel_spmd(nc, in_maps, core_ids=list(range(8)), trace=False)
            break
        except Exception as e:  # transient NRT device flakes
            last_err = e
            msg = str(e)
            if "UNRECOVERABLE" not in msg and "UNAVAILABLE" not in msg:
                raise
            import time as _time

            import jax as _jax

            _time.sleep(5 * (_attempt + 1))
            try:
                _jax.clear_backends()
            except Exception:
                pass
    else:
        raise last_err

    _CACHE["last_res"] = res
    corr = (bv @ Wo + bo).astype(np.float32)  # softmax rows sum to 1
    out = np.empty((B, T, D), np.float32)
    for b in range(B):
        out[b] = res.results[2 * b]["out"] + res.results[2 * b + 1]["out"] + corr
    return out


if __name__ == "__main__":
    rng = np.random.default_rng(0)
    inputs = {
        "x": rng.standard_normal((B, T, D)).astype(np.float32),
        "Wq": (rng.standard_normal((D, D)) / 32).astype(np.float32),
        "bq": np.zeros(D, np.float32),
        "Wk": (rng.standard_normal((D, D)) / 32).astype(np.float32),
        "bk": np.zeros(D, np.float32),
        "Wv": (rng.standard_normal((D, D)) / 32).astype(np.float32),
        "bv": np.zeros(D, np.float32),
        "Wo": (rng.standard_normal((D, D)) / 32).astype(np.float32),
        "bo": np.zeros(D, np.float32),
    }
    o = kernel(**inputs)
    print("ran", o.shape, o.dtype, float(np.abs(o).max()))


# revision 42
# speedup vs baseline: 1.1775x; 1.1775x over previous
"""BD3LM block-diffusion decoder layer on 8 trn2 NeuronCores.

Sharding: core = 2*b + g  (b = batch 0..3, g = head-group 0..1, 8 heads each).
Each core: QKV projections for its batch/head-group, sparse BD3LM attention
(only ~80 of 256 score tiles per head), O-projection against its Wo row-slice.
Host: sums the two group partials per batch and adds the (bv @ Wo + bo)
correction (softmax rows sum to 1, so the v-bias contributes exactly bv @ Wo).

v3 structure:
  - everything bf16 on the PE (1 cyc/row at any free size; f32r is 4 cyc/row
    below N=256). PSUM accumulation stays f32. End-to-end error ~5e-3.
  - x resident in SBUF; phase A emitted PAIR-major (projections for head
    pair p over the full sequence), immediately followed by attention for
    those two heads. Attention is ACT(exp)-bound, projections are PE-bound:
    the Tile scheduler overlaps pair p's exps with pair p+1's projections.
  - per head a [*,128] stationary v-block: col 0 = ones (softmax denominator
    -> ctx row 0, where reciprocal_approx_fast + gpsimd partition_broadcast
    work - both require physical partition 0), cols 64..127 = v channels
    (ctx rows 64..127, base-64-aligned for the normalize multiply).
  - scores [k_tile, q_span] into [128,1024] PSUM; ONE exp per (head, half,
    j) span and one batched exp for the 8 block-diagonal tiles (ACT per-op
    overhead ~293ns).
  - PSUM budget: proj pool 2 banks + shared score/ctx pool 6 banks.
"""

import numpy as np

import concourse.bass as bass
import concourse.mybir as mybir
import concourse.tile as tile
from concourse import bacc
from concourse.bass_utils import run_bass_kernel_spmd

F32 = mybir.dt.float32
BF16 = mybir.dt.bfloat16
Act = mybir.ActivationFunctionType

B, T, D = 4, 2048, 1024
H, HD = 16, 64
L = T // 2           # 1024, length of each of [xt | x0]
BS = 4               # block size
G = 2                # head groups (cores per batch)
DG = D // G          # 512 channels per group
HG = H // G          # 8 heads per core
P = 128
NT = L // P          # 8 key/query tiles per half
KC = D // P          # 8 contraction chunks
DT4 = DG // P        # 4 output-partition tiles for qT/kT

REPEAT = 1  # loop whole computation inside the NEFF (timing experiments only)
DBG = False

_CACHE = {}


def _chunks512(a0, a1):
    """Split [a0, a1) at multiples of 512 (PSUM bank boundaries)."""
    out = []
    while a0 < a1:
        b1 = min(a1, (a0 // 512 + 1) * 512)
        out.append((a0, b1))
        a0 = b1
    return out


def _build():
    import concourse.tile_utils as tile_utils

    tile_utils.max_sbuf_usage = 204 * 1024

    nc = bacc.Bacc("TRN2", target_bir_lowering=False, debug=False, num_devices=8)

    xT = nc.dram_tensor("xT", [D, T], BF16, kind="ExternalInput").ap()
    wq = nc.dram_tensor("wq", [D, DG], BF16, kind="ExternalInput").ap()
    wk = nc.dram_tensor("wk", [D, DG], BF16, kind="ExternalInput").ap()
    wv = nc.dram_tensor("wv", [D, DG], BF16, kind="ExternalInput").ap()
    wo = nc.dram_tensor("wo", [DG, D], BF16, kind="ExternalInput").ap()
    bqs = nc.dram_tensor("bqs", [DG], F32, kind="ExternalInput").ap()
    bks = nc.dram_tensor("bks", [DG], F32, kind="ExternalInput").ap()
    msk = nc.dram_tensor("msk", [3, P, P], BF16, kind="ExternalInput").ap()
    out = nc.dram_tensor("out", [T, D], F32, kind="ExternalOutput").ap()

    dbg = {}
    if DBG:
        for nm, shp, dt in (
            ("dbg_qT", [P, DT4, T], BF16),
            ("dbg_kT", [P, DT4, T], BF16),
            ("dbg_v", [P, T // P, HG * 2 * HD], BF16),
            ("dbg_ctxT", [P, DT4, T], BF16),
        ):
            dbg[nm] = nc.dram_tensor(nm, shp, dt, kind="ExternalOutput").ap()

    views = dict(
        dbg=dbg,
        xT_v=xT.rearrange("(kc p) t -> p kc t", p=P),    # [128, 8, 2048]
        wq_v=wq.rearrange("(kc p) m -> p kc m", p=P),    # [128, 8, 512]
        wk_v=wk.rearrange("(kc p) m -> p kc m", p=P),
        wv_v=wv.rearrange("(kc p) m -> p kc m", p=P),
        wo_v=wo.rearrange("(cc p) n -> p cc n", p=P),    # [128, 4, 1024]
        msk=msk,
        out=out,
    )

    with tile.TileContext(nc) as tc:
        with tc.tile_pool(name="persist", bufs=1) as pers:
            st = dict(
                x_sb=pers.tile([P, KC, T], BF16, name="x_sb"),
                wq_sb=pers.tile([P, KC, DG], BF16, name="wq_sb"),
                wk_sb=pers.tile([P, KC, DG], BF16, name="wk_sb"),
                wv_sb=pers.tile([P, KC, DG], BF16, name="wv_sb"),
                wo_sb=pers.tile([P, DT4, D], BF16, name="wo_sb"),
                qT_sb=pers.tile([P, DT4, T], BF16, name="qT_sb"),
                kT_sb=pers.tile([P, DT4, T], BF16, name="kT_sb"),
                v_sb=pers.tile([P, T // P, HG * 2 * HD], BF16, name="v_sb"),
                ctxT_sb=pers.tile([P, DT4, T], BF16, name="ctxT_sb"),
                bq_sb=pers.tile([P, DT4], F32, name="bq_sb"),
                bk_sb=pers.tile([P, DT4], F32, name="bk_sb"),
                m_strict=pers.tile([P, P], BF16, name="m_strict"),
                m_incl=pers.tile([P, P], BF16, name="m_incl"),
                m_diag=pers.tile([P, P], BF16, name="m_diag"),
            )
            nc.sync.dma_start(st["bq_sb"], bqs.rearrange("(c p) -> p c", p=P))
            nc.sync.dma_start(st["bk_sb"], bks.rearrange("(c p) -> p c", p=P))
            nc.sync.dma_start(st["m_strict"], msk[0])
            nc.sync.dma_start(st["m_incl"], msk[1])
            nc.sync.dma_start(st["m_diag"], msk[2])
            nc.gpsimd.memset(st["v_sb"], 0.0)
            ones_v = st["v_sb"].rearrange("p t (h c) -> p (t h) c", c=2 * HD)[
                :, :, 0:1
            ]
            nc.vector.memset(ones_v, 1.0)

            for _rep in range(REPEAT):
                _phases(nc, tc, st, views)

    nc.compile()
    return nc


def _load(nc, st, views):
    """Stream weights + x into SBUF across both HWDGE engines (SP + ACT),
    (kc, slab)-split so the first matmuls unblock after ~1MB of traffic."""
    # critical pieces first (DMA queues are FIFO): x slab 0 + wq/wk pair 0,
    # kc-split 32KB chunks alternating across the two HWDGE engines
    for kc in range(KC):
        eng, eng2 = (nc.sync, nc.scalar) if kc % 2 == 0 else (nc.scalar, nc.sync)
        eng.dma_start(
            st["x_sb"][:, kc, 0:512], views["xT_v"][:, kc, 0:512]
        )
        eng2.dma_start(
            st["wq_sb"][:, kc, 0:P], views["wq_v"][:, kc, 0:P]
        )
    nc.scalar.dma_start(st["wk_sb"][:, :, 0:P], views["wk_v"][:, :, 0:P])
    nc.sync.dma_start(st["wv_sb"][:, :, 0:P], views["wv_v"][:, :, 0:P])
    for s in range(1, T // 512):
        for kc in range(KC):
            eng = nc.sync if (kc % 2 == 0) else nc.scalar
            eng.dma_start(
                st["x_sb"][:, kc, 512 * s : 512 * (s + 1)],
                views["xT_v"][:, kc, 512 * s : 512 * (s + 1)],
            )
    for d4 in range(1, DT4):
        nc.sync.dma_start(
            st["wq_sb"][:, :, P * d4 : P * (d4 + 1)],
            views["wq_v"][:, :, P * d4 : P * (d4 + 1)],
        )
        nc.scalar.dma_start(
            st["wk_sb"][:, :, P * d4 : P * (d4 + 1)],
            views["wk_v"][:, :, P * d4 : P * (d4 + 1)],
        )
        nc.sync.dma_start(
            st["wv_sb"][:, :, P * d4 : P * (d4 + 1)],
            views["wv_v"][:, :, P * d4 : P * (d4 + 1)],
        )
    nc.scalar.dma_start(st["wo_sb"], views["wo_v"])


def _proj_pair(nc, st, pp, p):
    """QK projections for head pair p (qT/kT column-tile p); all of v is
    emitted with pair 0 (x-stationary N=512 matmuls cover every head)."""
    x_sb, v_sb = st["x_sb"], st["v_sb"]
    for w_sb, b_key, dst, scale in (
        (st["wq_sb"], "bq_sb", st["qT_sb"], HD ** -0.5),
        (st["wk_sb"], "bk_sb", st["kT_sb"], 1.0),
    ):
        for s in range(T // 512):
            ps = pp.tile([P, 512], F32, tag="pp", name=f"pp{p}_{s}")
            for kc in range(KC):
                nc.tensor.matmul(
                    ps,
                    w_sb[:, kc, P * p : P * (p + 1)],
                    x_sb[:, kc, 512 * s : 512 * (s + 1)],
                    start=(kc == 0),
                    stop=(kc == KC - 1),
                )
            nc.scalar.activation(
                dst[:, p, 512 * s : 512 * (s + 1)],
                ps,
                Act.Identity,
                bias=st[b_key][:, p : p + 1],
                scale=scale,
            )
    for tt in range(T // P):
        ps = pp.tile([P, 512], F32, tag="pp", name=f"ppv{p}_{tt}")
        for kc in range(KC):
            nc.tensor.matmul(
                ps[:, :P],
                x_sb[:, kc, P * tt : P * (tt + 1)],
                st["wv_sb"][:, kc, P * p : P * (p + 1)],
                start=(kc == 0),
                stop=(kc == KC - 1),
            )
        nc.vector.tensor_copy(
            v_sb[:, tt].rearrange("p (h c) -> p h c", c=2 * HD)[
                :, 2 * p : 2 * p + 2, HD : 2 * HD
            ],
            ps[:, :P].rearrange("p (h c) -> p h c", c=HD),
        )


def _attn_head(nc, st, bpool, atpool, tmppool, h):
    """Sparse BD3LM attention + normalize for one head."""
    qT_sb, kT_sb, v_sb = st["qT_sb"], st["kT_sb"], st["v_sb"]
    c, p0 = h // 2, HD * (h % 2)
    qh = qT_sb[p0 : p0 + HD, c, :]
    kh = kT_sb[p0 : p0 + HD, c, :]
    for half in range(2):
        mask = st["m_strict"] if half == 0 else st["m_incl"]
        ctx = bpool.tile([P, L], F32, tag="ps2", name=f"ctx{h}_{half}")
        for j in range(NT):
            span0 = P * j
            n = L - span0
            sc = bpool.tile([P, 1024], F32, tag="ps2", name=f"sc{h}_{half}_{j}")
            kv = kh[:, L + span0 : L + span0 + P]
            for r0, r1 in _chunks512(0, n):
                nc.tensor.matmul(
                    sc[:, r0:r1],
                    kv,
                    qh[:, L * half + span0 + r0 : L * half + span0 + r1],
                    start=True,
                    stop=True,
                    tile_position=(p0, 0),
                )
            at = atpool.tile([P, 1024], BF16, tag="at", name=f"at{h}_{half}_{j}")
            nc.scalar.activation(at[:, :n], sc[:, :n], Act.Exp)
            nc.vector.tensor_mul(at[:, :P], at[:, :P], mask)
            vj = v_sb[:, NT + j, 2 * HD * h : 2 * HD * (h + 1)]
            for a0, a1 in _chunks512(span0, L):
                last = half == 1 and (
                    (a1 <= 512 and j == 3) or (a0 >= 512 and j == NT - 1)
                )
                nc.tensor.matmul(
                    ctx[:, a0:a1],
                    vj,
                    at[:, a0 - span0 : a1 - span0],
                    start=(j == 0),
                    stop=last,
                )
        if half == 0:
            # xt-xt block-diagonal tiles, batched exp + mask
            scd = bpool.tile([P, 1024], F32, tag="ps2", name=f"scd{h}")
            for i in range(NT):
                nc.tensor.matmul(
                    scd[:, P * i : P * (i + 1)],
                    kh[:, P * i : P * (i + 1)],
                    qh[:, P * i : P * (i + 1)],
                    start=True,
                    stop=True,
                    tile_position=(p0, 0),
                )
            atd = atpool.tile([P, 1024], BF16, tag="at", name=f"atd{h}")
            nc.scalar.activation(atd, scd, Act.Exp)
            nc.vector.tensor_mul(
                atd.rearrange("p (i q) -> p i q", q=P),
                atd.rearrange("p (i q) -> p i q", q=P),
                st["m_diag"][:, None, :].to_broadcast((P, NT, P)),
            )
            for i in range(NT):
                nc.tensor.matmul(
                    ctx[:, P * i : P * (i + 1)],
                    v_sb[:, i, 2 * HD * h : 2 * HD * (h + 1)],
                    atd[:, P * i : P * (i + 1)],
                    start=False,
                    stop=(i == 3 or i == NT - 1),
                )
        # normalize per 512-bank: bank0's denominator row is final while
        # j=4..7 (resp. diag 4..7) still accumulate bank1, so its
        # recip/broadcast/multiply chain overlaps the remaining matmuls.
        cs = tmppool.tile([P, L], BF16, tag="cs", bufs=3, name=f"cs{h}_{half}")
        for c0 in range(0, L, 512):
            recip = tmppool.tile(
                [1, 512], F32, tag="recip", bufs=4, name=f"rc{h}_{half}_{c0}"
            )
            nc.vector.reciprocal_approx_fast(recip, ctx[0:1, c0 : c0 + 512])
            rb = tmppool.tile(
                [P, 512], F32, tag="rb", bufs=4, name=f"rb{h}_{half}_{c0}"
            )
            nc.gpsimd.partition_broadcast(rb, recip, channels=P)
            if p0 == HD:
                # odd head: ctxT target partitions 64..127 match the mul's
                # natural output partitions - write SBUF directly, no DMA
                nc.vector.tensor_mul(
                    st["ctxT_sb"][
                        p0 : p0 + HD, c, L * half + c0 : L * half + c0 + 512
                    ],
                    ctx[HD:P, c0 : c0 + 512],
                    rb[HD:P, :],
                )
            else:
                nc.vector.tensor_mul(
                    cs[HD:P, c0 : c0 + 512],
                    ctx[HD:P, c0 : c0 + 512],
                    rb[HD:P, :],
                )
                nc.sync.dma_start(
                    st["ctxT_sb"][
                        p0 : p0 + HD, c, L * half + c0 : L * half + c0 + 512
                    ],
                    cs[HD:P, c0 : c0 + 512],
                )


def _phases(nc, tc, st, views):
    from contextlib import ExitStack as _ES

    _load(nc, st, views)

    with tc.tile_pool(name="tmppool", bufs=2) as tmppool:
        _es = _ES()
        atpool = _es.enter_context(tc.tile_pool(name="atpool", bufs=8))
        pp = _es.enter_context(tc.tile_pool(name="pp", bufs=2, space="PSUM"))
        bpool = _es.enter_context(tc.tile_pool(name="bpool", bufs=3, space="PSUM"))

        for p in range(DT4):
            _proj_pair(nc, st, pp, p)
            _attn_head(nc, st, bpool, atpool, tmppool, 2 * p)
            _attn_head(nc, st, bpool, atpool, tmppool, 2 * p + 1)

        if DBG:
            nc.sync.dma_start(views["dbg"]["dbg_qT"], st["qT_sb"])
            nc.sync.dma_start(views["dbg"]["dbg_kT"], st["kT_sb"])
            nc.sync.dma_start(views["dbg"]["dbg_v"], st["v_sb"])
            nc.sync.dma_start(views["dbg"]["dbg_ctxT"], st["ctxT_sb"])

        _es.close()

        # ---------------- Phase C: O-projection ----------------
        with tc.tile_pool(name="opsum", bufs=6, space="PSUM") as opsum:
            for tt in range(T // P):
                for nk in range(2):
                    ops = opsum.tile([P, 512], F32, tag="op", name=f"op{tt}_{nk}")
                    for cc in range(DT4):
                        nc.tensor.matmul(
                            ops,
                            st["ctxT_sb"][:, cc, P * tt : P * (tt + 1)],
                            st["wo_sb"][:, cc, 512 * nk : 512 * (nk + 1)],
                            start=(cc == 0),
                            stop=(cc == DT4 - 1),
                        )
                    osb = tmppool.tile(
                        [P, 512], F32, tag="osb", bufs=6, name=f"osb{tt}_{nk}"
                    )
                    nc.vector.tensor_copy(osb, ops)
                    nc.sync.dma_start(
                        views["out"][
                            P * tt : P * (tt + 1), 512 * nk : 512 * (nk + 1)
                        ],
                        osb,
                    )


def _masks():
    import ml_dtypes

    q = np.arange(P)[None, :] // BS
    k = np.arange(P)[:, None] // BS
    m = np.zeros((3, P, P), np.float32)
    m[0] = (q > k).astype(np.float32)    # strict (xt q vs x0 k, same tile)
    m[1] = (q >= k).astype(np.float32)   # incl (x0 q vs x0 k, same tile)
    m[2] = (q == k).astype(np.float32)   # diag (xt q vs xt k, same tile)
    return m.astype(ml_dtypes.bfloat16)


def kernel(x, Wq, bq, Wk, bk, Wv, bv, Wo, bo, block_size=4, **_):
    import ml_dtypes

    BF = ml_dtypes.bfloat16
    x = np.asarray(x, np.float32)
    Wq, bq = np.asarray(Wq, np.float32), np.asarray(bq, np.float32)
    Wk, bk = np.asarray(Wk, np.float32), np.asarray(bk, np.float32)
    Wv, bv = np.asarray(Wv, np.float32), np.asarray(bv, np.float32)
    Wo, bo = np.asarray(Wo, np.float32), np.asarray(bo, np.float32)

    if "nc" not in _CACHE:
        _CACHE["nc"] = _build()
    nc = _CACHE["nc"]

    masks = _masks()
    scale = HD ** -0.5
    in_maps = []
    for core in range(8):
        b, g = core // 2, core % 2
        cols = slice(DG * g, DG * (g + 1))
        in_maps.append(
            {
                "xT": np.ascontiguousarray(x[b].T).astype(BF),
                "wq": np.ascontiguousarray(Wq[:, cols]).astype(BF),
                "wk": np.ascontiguousarray(Wk[:, cols]).astype(BF),
                "wv": np.ascontiguousarray(Wv[:, cols]).astype(BF),
                "wo": np.ascontiguousarray(Wo[cols, :]).astype(BF),
                "bqs": np.ascontiguousarray(bq[cols]) * np.float32(scale),
                "bks": np.ascontiguousarray(bk[cols]),
                "msk": masks,
            }
        )

    _CACHE["last_in_maps"] = in_maps
    last_err = None
    for _attempt in range(6):
        try:
            res = run_bass_kernel_spmd(nc, in_maps, core_ids=list(range(8)), trace=False)
            break
        except Exception as e:  # transient NRT device flakes
            last_err = e
            msg = str(e)
            if "UNRECOVERABLE" not in msg and "UNAVAILABLE" not in msg:
                raise
            import time as _time

            import jax as _jax

            _time.sleep(5 * (_attempt + 1))
            try:
                _jax.clear_backends()
            except Exception:
                pass
    else:
        raise last_err

    _CACHE["last_res"] = res
    corr = (bv @ Wo + bo).astype(np.float32)  # softmax rows sum to 1
    out = np.empty((B, T, D), np.float32)
    for b in range(B):
        out[b] = res.results[2 * b]["out"] + res.results[2 * b + 1]["out"] + corr
    return out


if __name__ == "__main__":
    rng = np.random.default_rng(0)
    inputs = {
        "x": rng.standard_normal((B, T, D)).astype(np.float32),
        "Wq": (rng.standard_normal((D, D)) / 32).astype(np.float32),
        "bq": np.zeros(D, np.float32),
        "Wk": (rng.standard_normal((D, D)) / 32).astype(np.float32),
        "bk": np.zeros(D, np.float32),
        "Wv": (rng.standard_normal((D, D)) / 32).astype(np.float32),
        "bv": np.zeros(D, np.float32),
        "Wo": (rng.standard_normal((D, D)) / 32).astype(np.float32),
        "bo": np.zeros(D, np.float32),
    }
    o = kernel(**inputs)
    print("ran", o.shape, o.dtype, float(np.abs(o).max()))
